# revision 33
# baseline (speedup 1.0000x reference)
"""AdaptiveFeaturePooling (2-level FPN ROI-align, adaptive sampling, summed)
as a Trainium2 Bass kernel on 8 NeuronCores.

Strategy
--------
The reference roi_align (sampling_ratio=-1, aligned=False, smax=2) is exactly
separable per ROI and level:

    out[r,c,py,px] = sum_lvl sum_{y,x} Wy[r,py,y] * Wx[r,px,x] * feat[b_r,c,y,x]

with Wy/Wx computable from boxes alone (1/count folded in).  The host
precomputes, for every ROI, the list of contributing feature pixels (both
levels concatenated) and packs pixel-value rows (256 ch) + per-pixel wy/wx
rows into dense bf16 arrays, as 128-row chunk-columns.  Each ROI's tail
(k mod 128 rows, rounded up to 32) is bin-packed into 32-grain sub-slots of
columns SHARED between ROIs; sub-chunk matmuls use partition offsets
(tile_position row groups), so the padding bytes are never transferred.

Each ROI on device is a PSUM-accumulated chain of matmuls with the VALUES as
the (FWL-eligible, 128-column) stationary operand and on-device-built
outer-product weights W2[pix,196] as the moving operand:

    psum[128ch_half, 196] += vals_chunk[pix, 128ch].T @ W2_chunk[pix, 196]

(the two channel halves accumulate in different PSUM banks).  W2 is built by
DVE/GpSimd from wy/wx row vectors; wy is pair-duplicated on device so the
multiply's innermost strides are all +-1 and the DVE 2x 16-bit perf mode
triggers.  ROIs are sharded across the 8 cores round-robin by descending
size so every core executes an identical instruction stream (SPMD).  Output
[256, 196] per ROI is copied to SBUF as bf16, DMA'd out, and the host does
the final layout fix + f32 cast while unsharding.
"""

import numpy as np
import ml_dtypes

P_OUT = 14
S_MAX = 2
N_CORES = 8
R_TOTAL = 512
C_FEAT = 256
CHUNK_K = 128
LEVELS = ((128, 0.25), (64, 0.125))  # (feature size, spatial_scale)

BF16_NP = ml_dtypes.bfloat16

GROUP = 8  # ROIs per DMA group


# ---------------------------------------------------------------- host math
def _axis_weights(lo, hi, size):
    """Separable 1D ROI-align weights for one axis of one level.

    lo/hi: f32 [R] box edges in feature coords. Returns (W [R,14,size] f32
    with 1/g folded in, support lo index [R], support hi index [R])."""
    roi = np.maximum(hi - lo, np.float32(1.0))
    bin_ = roi / np.float32(P_OUT)
    g = np.ceil(roi / P_OUT).astype(np.int32)
    gf = g.astype(np.float32)
    p = np.arange(P_OUT, dtype=np.float32)
    s = np.arange(S_MAX, dtype=np.float32)
    coord = (
        lo[:, None, None]
        + p[None, :, None] * bin_[:, None, None]
        + (s[None, None, :] + np.float32(0.5)) * (bin_ / gf)[:, None, None]
    )
    smask = np.arange(S_MAX)[None, :] < g[:, None]
    valid = (coord >= np.float32(-1.0)) & (coord <= np.float32(size))
    m = smask[:, None, :] & valid
    cc = np.clip(coord, np.float32(0.0), np.float32(size - 1))
    c0 = np.floor(cc).astype(np.int32)
    c1 = np.minimum(c0 + 1, size - 1)
    l = cc - c0.astype(np.float32)
    h = np.float32(1.0) - l
    wl = np.where(m, h, np.float32(0.0))
    wh = np.where(m, l, np.float32(0.0))
    R = lo.shape[0]
    W = np.zeros((R, P_OUT, size), np.float32)
    ridx = np.arange(R)[:, None, None]
    pidx = np.arange(P_OUT)[None, :, None]
    np.add.at(W, (ridx, pidx, c0), wl)
    np.add.at(W, (ridx, pidx, c1), wh)
    W /= gf[:, None, None]
    any_col = W.any(axis=1)
    has = any_col.any(axis=1)
    lo_i = np.where(has, np.argmax(any_col, axis=1), 0).astype(np.int64)
    hi_i = np.where(has, size - 1 - np.argmax(any_col[:, ::-1], axis=1), 0).astype(
        np.int64
    )
    return W, lo_i, hi_i


def _build_rois(boxes, batch_idx):
    """Per-ROI concatenated (level, y, x) pixel lists + wy/wx rows."""
    per_level = []
    for size, scale in LEVELS:
        b = boxes * np.float32(scale)
        Wx, xlo, xhi = _axis_weights(b[:, 0], b[:, 2], size)
        Wy, ylo, yhi = _axis_weights(b[:, 1], b[:, 3], size)
        per_level.append((Wy, ylo, yhi, Wx, xlo, xhi))

    rois = []
    for r in range(boxes.shape[0]):
        lvls, ys_l, xs_l, wys, wxs = [], [], [], [], []
        for lvl, (Wy, ylo, yhi, Wx, xlo, xhi) in enumerate(per_level):
            ys = np.arange(ylo[r], yhi[r] + 1)
            xs = np.arange(xlo[r], xhi[r] + 1)
            if ys.size == 0 or xs.size == 0:
                continue
            WyS = Wy[r][:, ys]  # [14, Hr]
            WxS = Wx[r][:, xs]  # [14, Wr]
            yy, xx = np.meshgrid(ys, xs, indexing="ij")
            lvls.append(np.full(yy.size, lvl, np.int64))
            ys_l.append(yy.ravel())
            xs_l.append(xx.ravel())
            yloc = (yy - ys[0]).ravel()
            xloc = (xx - xs[0]).ravel()
            wys.append(WyS.T[yloc])  # [K, 14]
            wxs.append(WxS.T[xloc])  # [K, 14]
        if lvls:
            lvl_a = np.concatenate(lvls)
            y_a = np.concatenate(ys_l)
            x_a = np.concatenate(xs_l)
            wy_a = np.concatenate(wys, axis=0).astype(np.float32)
            wx_a = np.concatenate(wxs, axis=0).astype(np.float32)
        else:
            lvl_a = np.zeros(1, np.int64)
            y_a = np.zeros(1, np.int64)
            x_a = np.zeros(1, np.int64)
            wy_a = np.zeros((1, P_OUT), np.float32)
            wx_a = np.zeros((1, P_OUT), np.float32)
        rois.append(
            dict(
                b=int(batch_idx[r]),
                lvl=lvl_a,
                y=y_a,
                x=x_a,
                wy=wy_a,
                wx=wx_a,
                k=lvl_a.size,
            )
        )
    return rois


# ------------------------------------------------------------- SPMD layout
def _build_layout(slot_k):
    """slot_k: [n_slots] max-over-cores pixel count per slot (group-ordered).

    Returns a per-group layout.  Values (regs): full chunks of each slot
    occupy dedicated consecutive columns; 32-grain tails are first-fit
    bin-packed (descending) into columns SHARED between slots.  Weights
    (wyx): full chunks mirror the regs columns; each slot's tail gets a
    DEDICATED wyx column, zero outside its [toff, toff+th) rows, so tail
    matmuls can contract all 128 partitions (other slots' value rows are
    multiplied by zero weights) and stay on the fast full-row PE path."""
    n_slots = len(slot_k)
    n_groups = n_slots // GROUP
    groups = []
    for g in range(n_groups):
        ks = slot_k[g * GROUP : (g + 1) * GROUP]
        nf = [k // CHUNK_K for k in ks]
        th = [-(-(k - f * CHUNK_K) // 32) * 32 for k, f in zip(ks, nf)]
        # full-height tails become full chunks
        for i in range(GROUP):
            if th[i] == CHUNK_K:
                nf[i] += 1
                th[i] = 0
        fstart = []
        c = 0
        for i in range(GROUP):
            fstart.append(c)
            c += nf[i]
        # first-fit-decreasing tail packing into shared regs columns
        order = sorted(range(GROUP), key=lambda i: -th[i])
        tcols = []  # used height per shared tail column
        tassign = {}
        for i in order:
            if th[i] == 0:
                continue
            placed = False
            for ci, used in enumerate(tcols):
                if used + th[i] <= CHUNK_K:
                    tassign[i] = (ci, used)
                    tcols[ci] += th[i]
                    placed = True
                    break
            if not placed:
                tassign[i] = (len(tcols), 0)
                tcols.append(th[i])
        # dedicated wyx tail columns, in slot order after the full columns
        slots = []
        wc = c
        for i in range(GROUP):
            if th[i] > 0:
                ci, off = tassign[i]
                slots.append((nf[i], fstart[i], c + ci, off, th[i], wc))
                wc += 1
            else:
                slots.append((nf[i], fstart[i], -1, 0, 0, -1))
        groups.append(
            dict(slots=slots, ncols=c + len(tcols), wcols=wc, nfull=c)
        )
    return groups


# ---------------------------------------------------------------- device graph
_GRAPH_CACHE = {}

# measured in-kernel per-chunk (196 elem/lane) outer-product cost and
# per-column wy pair-duplication cost, ns
_TT_V, _TT_G = 175.0, 515.0
_WYD_V, _WYD_G = 35.0, 90.0


def _build_graph(layout_key):
    if layout_key in _GRAPH_CACHE:
        return _GRAPH_CACHE[layout_key]

    import concourse.bass as bass
    import concourse.bacc as bacc
    import concourse.tile as tile
    import concourse.mybir as mybir

    BF16 = mybir.dt.bfloat16
    F32 = mybir.dt.float32

    groups = [
        dict(slots=s, ncols=nc_, wcols=wc, nfull=nfu)
        for (s, nc_, wc, nfu) in layout_key
    ]
    n_groups = len(groups)
    tot = sum(g["ncols"] for g in groups)
    totw = sum(g["wcols"] for g in groups)

    nc = bacc.Bacc(
        "TRN2", target_bir_lowering=False, debug=False, enable_asserts=False
    )
    regs = nc.declare_dram_parameter("regs", [128, tot, 256], BF16, isOutput=False)
    wyxs = nc.declare_dram_parameter("wyxs", [128, totw, 28], BF16, isOutput=False)
    out = nc.declare_dram_parameter(
        "out", [n_groups, 128, GROUP * 392], BF16, isOutput=True
    )

    # greedy DVE/gpsimd balance for outer-product + wyd work (ns)
    load = {"v": 0.0, "g": 0.0}

    def pick(cost_v, cost_g):
        if load["v"] + cost_v <= load["g"] + cost_g:
            load["v"] += cost_v
            return "v"
        load["g"] += cost_g
        return "g"

    with tile.TileContext(nc) as tc:
        with (
            tc.tile_pool(name="data", bufs=5) as data_pool,
            tc.tile_pool(name="wyxp", bufs=n_groups) as wyx_pool,
            tc.tile_pool(name="wydp", bufs=3) as wyd_pool,
            tc.tile_pool(name="w2p", bufs=16) as w2_pool,
            tc.tile_pool(name="psum", bufs=4, space="PSUM") as psum_pool,
            tc.tile_pool(name="outp", bufs=4) as out_pool,
        ):
            # ALL wyx loads issued up front: group 0's on the sync ring
            # (lands before the big reg loads), the rest on the ACT ring.
            # Issuing them inside the group loop would queue each one behind
            # the previous group's PSUM copies in the ACT program order,
            # stalling the outer-product engines mid-kernel.
            wyx_tiles = []
            wstart = 0
            for g, G in enumerate(groups):
                gwc = G["wcols"]
                wyx_t = wyx_pool.tile([128, gwc * 28], BF16, tag="wyx")
                (nc.sync if g == 0 else nc.scalar).dma_start(
                    wyx_t[:],
                    wyxs[:, wstart : wstart + gwc, :].rearrange(
                        "p t c -> p (t c)"
                    ),
                )
                wyx_tiles.append(wyx_t)
                wstart += gwc

            start = 0
            for g, G in enumerate(groups):
                gcc = G["ncols"]
                gwc = G["wcols"]
                reg_t = data_pool.tile([128, gcc * 256], BF16, tag="reg")
                wyx_t = wyx_tiles[g]
                wyd_t = wyd_pool.tile([128, gwc * 28], BF16, tag="wyd")
                # reg load split for finer-grained compute overlap.  For
                # group 0 the second half goes through gpsimd's SWDGE (idle
                # at kernel start): descriptor-gen runs in parallel with the
                # sync ring's, so the DMA window opens earlier and group-1's
                # load descriptors start generating sooner.
                nsplit = 2
                bounds = sorted(
                    {min(gcc, (i * gcc + nsplit - 1) // nsplit) for i in
                     range(nsplit + 1)}
                )
                for si, (a, b) in enumerate(zip(bounds, bounds[1:])):
                    if a == b:
                        continue
                    eng_dma = nc.gpsimd if (g == 0 and si == 1) else nc.sync
                    eng_dma.dma_start(
                        reg_t[:, a * 256 : b * 256],
                        regs[:, start + a : start + b, :].rearrange(
                            "p t c -> p (t c)"
                        ),
                    )
                # pair-duplicate wy: wyd[p, c, a, d=2] = wy[p, c, a] so the
                # outer-product's innermost strides are all +-1 (DVE 2x mode)
                wyx_g = wyx_t[:].rearrange("p (t z) -> p t z", t=gwc)
                eng = pick(gwc * _WYD_V, gwc * _WYD_G)
                (nc.vector if eng == "v" else nc.gpsimd).tensor_copy(
                    wyd_t[:].rearrange("p (t a d) -> p t a d", t=gwc, a=P_OUT),
                    wyx_g[:, :, 0:14][:, :, :, None].broadcast_to(
                        [128, gwc, P_OUT, 2]
                    ),
                )

                def tt(w2_ap, c0, ncc):
                    """w2[p, t, a, b] = wy[p, t, a] * wx[p, t, b] for columns
                    [c0, c0+ncc) of this group's wyx/wyd tiles."""
                    wyd_r = wyd_t[:, c0 * 28 : (c0 + ncc) * 28].rearrange(
                        "p (t a d) -> p t a d", t=ncc, a=P_OUT
                    )
                    wx_pair = wyx_t[:, c0 * 28 : (c0 + ncc) * 28].rearrange(
                        "p (t z) -> p t z", t=ncc
                    )[:, :, 14:28].rearrange("p t (b2 d) -> p t b2 d", d=2)
                    eng = pick(ncc * _TT_V, ncc * _TT_G)
                    (nc.vector if eng == "v" else nc.gpsimd).tensor_mul(
                        w2_ap.rearrange(
                            "p (t a b2 d) -> p t a b2 d", t=ncc, a=P_OUT, b2=7
                        ),
                        wyd_r[:, :, :, None, :].broadcast_to(
                            [128, ncc, P_OUT, 7, 2]
                        ),
                        wx_pair[:, :, None, :, :].broadcast_to(
                            [128, ncc, P_OUT, 7, 2]
                        ),
                    )

                ot = out_pool.tile([128, GROUP * 392], BF16)
                for i, (nf, fstart, tcol, toff, th, wtail) in enumerate(
                    G["slots"]
                ):
                    if nf > 0:
                        w2f = w2_pool.tile([128, nf * 196], BF16, tag="w2")
                        tt(w2f[:], fstart, nf)
                    if th > 0:
                        # tail: dedicated zero-masked wyx column -> full
                        # 128-row contraction against the SHARED regs column
                        w2t = w2_pool.tile([128, 196], BF16, tag="w2")
                        tt(w2t[:], wtail, 1)
                    ps = psum_pool.tile([128, 1024], F32)
                    # channel halves accumulate in DIFFERENT PSUM banks: a
                    # matmul's start=True clears the whole bank, so
                    # interleaved chains sharing a bank corrupt each other
                    for t in range(nf):
                        for h in range(2):
                            nc.tensor.matmul(
                                ps[:, h * 512 : h * 512 + 196],
                                reg_t[
                                    :,
                                    (fstart + t) * 256
                                    + h * 128 : (fstart + t) * 256
                                    + (h + 1) * 128,
                                ],
                                w2f[:, t * 196 : (t + 1) * 196],
                                start=(t == 0),
                                stop=(t == nf - 1 and th == 0),
                            )
                    if th > 0:
                        for h in range(2):
                            nc.tensor.matmul(
                                ps[:, h * 512 : h * 512 + 196],
                                reg_t[
                                    :,
                                    tcol * 256 + h * 128 : tcol * 256
                                    + (h + 1) * 128,
                                ],
                                w2t[:, :],
                                start=(nf == 0),
                                stop=True,
                            )
                    ps_view = ps[:].rearrange("p (h z) -> p h z", h=2)[:, :, 0:196]
                    ot_view = ot[:, i * 392 : (i + 1) * 392].rearrange(
                        "p (h z) -> p h z", h=2
                    )
                    # last group's copies split ACT/DVE so the drain chain
                    # (copies -> final stores) finishes sooner
                    if g == n_groups - 1 and i % 2 == 1:
                        nc.vector.tensor_copy(ot_view, ps_view)
                    else:
                        nc.scalar.copy(ot_view, ps_view)
                # mid stores on the scalar HWDGE ring: they sit right after
                # the copies they depend on in ACT program order, so the
                # sequencer never stalls on them (on the sync ring a store's
                # semaphore wait would block the NEXT group's load
                # descriptor-gen and collapse the pipeline).  Tail stores on
                # sync (idle once loads are done).  2 per group max — each
                # dma_start costs ~0.65us of serial sequencer descriptor-gen.
                if g >= n_groups - 2:
                    half = GROUP // 2 * 392
                    nc.sync.dma_start(out[g][:, :half], ot[:, :half])
                    nc.sync.dma_start(out[g][:, half:], ot[:, half:])
                else:
                    nc.scalar.dma_start(out[g], ot[:])
                start += gcc
    nc.compile()
    _GRAPH_CACHE[layout_key] = nc
    return nc


# ---------------------------------------------------------------- entry point
def _run(feature_f4, feature_f8, boxes, batch_idx, trace=False):
    from concourse.bass_utils import run_bass_kernel_spmd

    feature_f4 = np.ascontiguousarray(np.asarray(feature_f4, dtype=np.float32))
    feature_f8 = np.ascontiguousarray(np.asarray(feature_f8, dtype=np.float32))
    boxes = np.asarray(boxes, dtype=np.float32)
    batch_idx = np.asarray(batch_idx)

    rois = _build_rois(boxes, batch_idx)
    R = len(rois)
    assert R % N_CORES == 0
    n_slots = R // N_CORES
    n_groups = n_slots // GROUP

    # shard: descending size, round-robin deal => slot rank j holds ROIs of
    # rank j*8..j*8+7, so per-slot sizes are near-equal across cores
    order = sorted(range(R), key=lambda r: (-rois[r]["k"], r))
    assign = [order[c::N_CORES] for c in range(N_CORES)]  # [core][rank] -> roi
    # snake-deal slots so every GROUP mixes large and small ROIs: BALANCED
    # group sizes keep the PE continuously fed (skewed groups starve it
    # mid-kernel and trip HAM re-throttling)
    slot_order = [g + n_groups * i for g in range(n_groups) for i in range(GROUP)]
    # order groups: 2nd-smallest first (fast ramp-in), smallest LAST (fast
    # pipeline drain)
    gw = []
    for g in range(n_groups):
        members = slot_order[g * GROUP : (g + 1) * GROUP]
        gw.append((sum(rois[assign[0][s]]["k"] for s in members), g))
    asc = [g for _, g in sorted(gw)]
    g_order = [asc[1]] + asc[2:][::-1] + [asc[0]]
    slot_order = [
        s for g in g_order for s in slot_order[g * GROUP : (g + 1) * GROUP]
    ]
    assign = [[a[s] for s in slot_order] for a in assign]

    slot_k = [
        max(rois[assign[c][j]]["k"] for c in range(N_CORES))
        for j in range(n_slots)
    ]
    layout = _build_layout(slot_k)
    layout_key = tuple(
        (tuple(G["slots"]), G["ncols"], G["wcols"], G["nfull"])
        for G in layout
    )
    tot = sum(G["ncols"] for G in layout)
    totw = sum(G["wcols"] for G in layout)

    # NHWC bf16 feature copies for row gathering
    feats_bf = [
        np.ascontiguousarray(feature_f4.transpose(0, 2, 3, 1)).astype(BF16_NP),
        np.ascontiguousarray(feature_f8.transpose(0, 2, 3, 1)).astype(BF16_NP),
    ]

    in_maps = []
    for c in range(N_CORES):
        regs_c = np.zeros((128, tot, 256), BF16_NP)
        wyxs_c = np.zeros((128, totw, 28), BF16_NP)
        base = 0
        wbase = 0
        for g, G in enumerate(layout):
            for i, (nf, fstart, tcol, toff, th, wtail) in enumerate(
                G["slots"]
            ):
                d = rois[assign[c][g * GROUP + i]]
                k = d["k"]
                vals = np.empty((k, 256), BF16_NP)
                for lvl in (0, 1):
                    sel = d["lvl"] == lvl
                    if sel.any():
                        vals[sel] = feats_bf[lvl][d["b"]][d["y"][sel], d["x"][sel]]
                wy_bf = d["wy"].astype(BF16_NP)
                wx_bf = d["wx"].astype(BF16_NP)

                def put(vcol, wcol, prow, a, b):
                    n = b - a
                    regs_c[prow : prow + n, base + vcol, :] = vals[a:b]
                    wyxs_c[prow : prow + n, wbase + wcol, 0:14] = wy_bf[a:b]
                    wyxs_c[prow : prow + n, wbase + wcol, 14:28] = wx_bf[a:b]

                for t in range(nf):
                    a, b = t * CHUNK_K, min((t + 1) * CHUNK_K, k)
                    if a >= k:
                        break
                    put(fstart + t, fstart + t, 0, a, b)
                if th > 0 and k > nf * CHUNK_K:
                    put(tcol, wtail, toff, nf * CHUNK_K, k)
            base += G["ncols"]
            wbase += G["wcols"]
        in_maps.append({"regs": regs_c, "wyxs": wyxs_c})

    nc = _build_graph(layout_key)
    res = run_bass_kernel_spmd(
        nc, in_maps, core_ids=list(range(N_CORES)), trace=trace
    )

    # unshard + layout fix
    out_full = np.empty((R, 256, P_OUT, P_OUT), np.float32)
    for c in range(N_CORES):
        o = res.results[c]["out"].astype(np.float32)  # [n_groups, 128, G*392]
        o = o.reshape(n_groups, 128, GROUP, 2, 196).transpose(0, 2, 3, 1, 4)
        o = o.reshape(n_slots, 256, 196)
        out_full[assign[c]] = o.reshape(n_slots, 256, P_OUT, P_OUT)
    return out_full, res


def kernel(feature_f4, feature_f8, boxes, batch_idx):
    out, _ = _run(feature_f4, feature_f8, boxes, batch_idx, trace=False)
    return out


# revision 34
# speedup vs baseline: 1.0062x; 1.0062x over previous
"""AdaptiveFeaturePooling (2-level FPN ROI-align, adaptive sampling, summed)
as a Trainium2 Bass kernel on 8 NeuronCores.

Strategy
--------
The reference roi_align (sampling_ratio=-1, aligned=False, smax=2) is exactly
separable per ROI and level:

    out[r,c,py,px] = sum_lvl sum_{y,x} Wy[r,py,y] * Wx[r,px,x] * feat[b_r,c,y,x]

with Wy/Wx computable from boxes alone (1/count folded in).  The host
precomputes, for every ROI, the list of contributing feature pixels (both
levels concatenated) and packs pixel-value rows (256 ch) + per-pixel wy/wx
rows into dense bf16 arrays, as 128-row chunk-columns.  Each ROI's tail
(k mod 128 rows, rounded up to 32) is bin-packed into 32-grain sub-slots of
columns SHARED between ROIs; sub-chunk matmuls use partition offsets
(tile_position row groups), so the padding bytes are never transferred.

Each ROI on device is a PSUM-accumulated chain of matmuls with the VALUES as
the (FWL-eligible, 128-column) stationary operand and on-device-built
outer-product weights W2[pix,196] as the moving operand:

    psum[128ch_half, 196] += vals_chunk[pix, 128ch].T @ W2_chunk[pix, 196]

(the two channel halves accumulate in different PSUM banks).  W2 is built by
DVE/GpSimd from wy/wx row vectors; wy is pair-duplicated on device so the
multiply's innermost strides are all +-1 and the DVE 2x 16-bit perf mode
triggers.  ROIs are sharded across the 8 cores round-robin by descending
size so every core executes an identical instruction stream (SPMD).  Output
[256, 196] per ROI is copied to SBUF as bf16, DMA'd out, and the host does
the final layout fix + f32 cast while unsharding.
"""

import numpy as np
import ml_dtypes

P_OUT = 14
S_MAX = 2
N_CORES = 8
R_TOTAL = 512
C_FEAT = 256
CHUNK_K = 128
LEVELS = ((128, 0.25), (64, 0.125))  # (feature size, spatial_scale)

BF16_NP = ml_dtypes.bfloat16

GROUP = 8  # ROIs per DMA group


# ---------------------------------------------------------------- host math
def _axis_weights(lo, hi, size):
    """Separable 1D ROI-align weights for one axis of one level.

    lo/hi: f32 [R] box edges in feature coords. Returns (W [R,14,size] f32
    with 1/g folded in, support lo index [R], support hi index [R])."""
    roi = np.maximum(hi - lo, np.float32(1.0))
    bin_ = roi / np.float32(P_OUT)
    g = np.ceil(roi / P_OUT).astype(np.int32)
    gf = g.astype(np.float32)
    p = np.arange(P_OUT, dtype=np.float32)
    s = np.arange(S_MAX, dtype=np.float32)
    coord = (
        lo[:, None, None]
        + p[None, :, None] * bin_[:, None, None]
        + (s[None, None, :] + np.float32(0.5)) * (bin_ / gf)[:, None, None]
    )
    smask = np.arange(S_MAX)[None, :] < g[:, None]
    valid = (coord >= np.float32(-1.0)) & (coord <= np.float32(size))
    m = smask[:, None, :] & valid
    cc = np.clip(coord, np.float32(0.0), np.float32(size - 1))
    c0 = np.floor(cc).astype(np.int32)
    c1 = np.minimum(c0 + 1, size - 1)
    l = cc - c0.astype(np.float32)
    h = np.float32(1.0) - l
    wl = np.where(m, h, np.float32(0.0))
    wh = np.where(m, l, np.float32(0.0))
    R = lo.shape[0]
    W = np.zeros((R, P_OUT, size), np.float32)
    ridx = np.arange(R)[:, None, None]
    pidx = np.arange(P_OUT)[None, :, None]
    np.add.at(W, (ridx, pidx, c0), wl)
    np.add.at(W, (ridx, pidx, c1), wh)
    W /= gf[:, None, None]
    any_col = W.any(axis=1)
    has = any_col.any(axis=1)
    lo_i = np.where(has, np.argmax(any_col, axis=1), 0).astype(np.int64)
    hi_i = np.where(has, size - 1 - np.argmax(any_col[:, ::-1], axis=1), 0).astype(
        np.int64
    )
    return W, lo_i, hi_i


def _build_rois(boxes, batch_idx):
    """Per-ROI concatenated (level, y, x) pixel lists + wy/wx rows."""
    per_level = []
    for size, scale in LEVELS:
        b = boxes * np.float32(scale)
        Wx, xlo, xhi = _axis_weights(b[:, 0], b[:, 2], size)
        Wy, ylo, yhi = _axis_weights(b[:, 1], b[:, 3], size)
        per_level.append((Wy, ylo, yhi, Wx, xlo, xhi))

    rois = []
    for r in range(boxes.shape[0]):
        lvls, ys_l, xs_l, wys, wxs = [], [], [], [], []
        for lvl, (Wy, ylo, yhi, Wx, xlo, xhi) in enumerate(per_level):
            ys = np.arange(ylo[r], yhi[r] + 1)
            xs = np.arange(xlo[r], xhi[r] + 1)
            if ys.size == 0 or xs.size == 0:
                continue
            WyS = Wy[r][:, ys]  # [14, Hr]
            WxS = Wx[r][:, xs]  # [14, Wr]
            yy, xx = np.meshgrid(ys, xs, indexing="ij")
            lvls.append(np.full(yy.size, lvl, np.int64))
            ys_l.append(yy.ravel())
            xs_l.append(xx.ravel())
            yloc = (yy - ys[0]).ravel()
            xloc = (xx - xs[0]).ravel()
            wys.append(WyS.T[yloc])  # [K, 14]
            wxs.append(WxS.T[xloc])  # [K, 14]
        if lvls:
            lvl_a = np.concatenate(lvls)
            y_a = np.concatenate(ys_l)
            x_a = np.concatenate(xs_l)
            wy_a = np.concatenate(wys, axis=0).astype(np.float32)
            wx_a = np.concatenate(wxs, axis=0).astype(np.float32)
        else:
            lvl_a = np.zeros(1, np.int64)
            y_a = np.zeros(1, np.int64)
            x_a = np.zeros(1, np.int64)
            wy_a = np.zeros((1, P_OUT), np.float32)
            wx_a = np.zeros((1, P_OUT), np.float32)
        rois.append(
            dict(
                b=int(batch_idx[r]),
                lvl=lvl_a,
                y=y_a,
                x=x_a,
                wy=wy_a,
                wx=wx_a,
                k=lvl_a.size,
            )
        )
    return rois


# ------------------------------------------------------------- SPMD layout
def _build_layout(slot_k):
    """slot_k: [n_slots] max-over-cores pixel count per slot (group-ordered).

    Returns a per-group layout.  Values (regs): full chunks of each slot
    occupy dedicated consecutive columns; 32-grain tails are first-fit
    bin-packed (descending) into columns SHARED between slots.  Weights
    (wyx): full chunks mirror the regs columns; each slot's tail gets a
    DEDICATED wyx column, zero outside its [toff, toff+th) rows, so tail
    matmuls can contract all 128 partitions (other slots' value rows are
    multiplied by zero weights) and stay on the fast full-row PE path."""
    n_slots = len(slot_k)
    n_groups = n_slots // GROUP
    groups = []
    for g in range(n_groups):
        ks = slot_k[g * GROUP : (g + 1) * GROUP]
        nf = [k // CHUNK_K for k in ks]
        th = [-(-(k - f * CHUNK_K) // 32) * 32 for k, f in zip(ks, nf)]
        # full-height tails become full chunks
        for i in range(GROUP):
            if th[i] == CHUNK_K:
                nf[i] += 1
                th[i] = 0
        fstart = []
        c = 0
        for i in range(GROUP):
            fstart.append(c)
            c += nf[i]
        # first-fit-decreasing tail packing into shared regs columns
        order = sorted(range(GROUP), key=lambda i: -th[i])
        tcols = []  # used height per shared tail column
        tassign = {}
        for i in order:
            if th[i] == 0:
                continue
            placed = False
            for ci, used in enumerate(tcols):
                if used + th[i] <= CHUNK_K:
                    tassign[i] = (ci, used)
                    tcols[ci] += th[i]
                    placed = True
                    break
            if not placed:
                tassign[i] = (len(tcols), 0)
                tcols.append(th[i])
        # dedicated wyx tail columns, in slot order after the full columns
        slots = []
        wc = c
        for i in range(GROUP):
            if th[i] > 0:
                ci, off = tassign[i]
                slots.append((nf[i], fstart[i], c + ci, off, th[i], wc))
                wc += 1
            else:
                slots.append((nf[i], fstart[i], -1, 0, 0, -1))
        groups.append(
            dict(slots=slots, ncols=c + len(tcols), wcols=wc, nfull=c)
        )
    return groups


# ---------------------------------------------------------------- device graph
_GRAPH_CACHE = {}

# measured in-kernel per-chunk (196 elem/lane) outer-product cost and
# per-column wy pair-duplication cost, ns
_TT_V, _TT_G = 175.0, 515.0
_WYD_V, _WYD_G = 35.0, 90.0


def _build_graph(layout_key):
    if layout_key in _GRAPH_CACHE:
        return _GRAPH_CACHE[layout_key]

    import concourse.bass as bass
    import concourse.bacc as bacc
    import concourse.tile as tile
    import concourse.mybir as mybir

    BF16 = mybir.dt.bfloat16
    F32 = mybir.dt.float32

    groups = [
        dict(slots=s, ncols=nc_, wcols=wc, nfull=nfu)
        for (s, nc_, wc, nfu) in layout_key
    ]
    n_groups = len(groups)
    tot = sum(g["ncols"] for g in groups)
    totw = sum(g["wcols"] for g in groups)

    nc = bacc.Bacc(
        "TRN2", target_bir_lowering=False, debug=False, enable_asserts=False
    )
    regs = nc.declare_dram_parameter("regs", [128, tot, 256], BF16, isOutput=False)
    wyxs = nc.declare_dram_parameter("wyxs", [128, totw, 28], BF16, isOutput=False)
    out = nc.declare_dram_parameter(
        "out", [n_groups, 128, GROUP * 392], BF16, isOutput=True
    )

    # greedy DVE/gpsimd balance for outer-product + wyd work (ns)
    load = {"v": 0.0, "g": 0.0}

    def pick(cost_v, cost_g):
        if load["v"] + cost_v <= load["g"] + cost_g:
            load["v"] += cost_v
            return "v"
        load["g"] += cost_g
        return "g"

    with tile.TileContext(nc) as tc:
        with (
            tc.tile_pool(name="data", bufs=5) as data_pool,
            tc.tile_pool(name="wyxp", bufs=n_groups) as wyx_pool,
            tc.tile_pool(name="wydp", bufs=3) as wyd_pool,
            tc.tile_pool(name="w2p", bufs=16) as w2_pool,
            tc.tile_pool(name="psum", bufs=4, space="PSUM") as psum_pool,
            tc.tile_pool(name="outp", bufs=4) as out_pool,
        ):
            # ALL wyx loads issued up front: group 0's on the sync ring
            # (lands before the big reg loads), the rest on the ACT ring.
            # Issuing them inside the group loop would queue each one behind
            # the previous group's PSUM copies in the ACT program order,
            # stalling the outer-product engines mid-kernel.
            wyx_tiles = []
            wstart = 0
            for g, G in enumerate(groups):
                gwc = G["wcols"]
                wyx_t = wyx_pool.tile([128, gwc * 28], BF16, tag="wyx")
                (nc.sync if g == 0 else nc.scalar).dma_start(
                    wyx_t[:],
                    wyxs[:, wstart : wstart + gwc, :].rearrange(
                        "p t c -> p (t c)"
                    ),
                )
                wyx_tiles.append(wyx_t)
                wstart += gwc

            start = 0
            for g, G in enumerate(groups):
                gcc = G["ncols"]
                gwc = G["wcols"]
                reg_t = data_pool.tile([128, gcc * 256], BF16, tag="reg")
                wyx_t = wyx_tiles[g]
                wyd_t = wyd_pool.tile([128, gwc * 28], BF16, tag="wyd")
                # reg load split for finer-grained compute overlap
                nsplit = 4 if g == 0 else 2
                bounds = sorted(
                    {min(gcc, (i * gcc + nsplit - 1) // nsplit) for i in
                     range(nsplit + 1)}
                )
                for a, b in zip(bounds, bounds[1:]):
                    if a == b:
                        continue
                    nc.sync.dma_start(
                        reg_t[:, a * 256 : b * 256],
                        regs[:, start + a : start + b, :].rearrange(
                            "p t c -> p (t c)"
                        ),
                    )
                # pair-duplicate wy: wyd[p, c, a, d=2] = wy[p, c, a] so the
                # outer-product's innermost strides are all +-1 (DVE 2x mode)
                wyx_g = wyx_t[:].rearrange("p (t z) -> p t z", t=gwc)
                eng = pick(gwc * _WYD_V, gwc * _WYD_G)
                (nc.vector if eng == "v" else nc.gpsimd).tensor_copy(
                    wyd_t[:].rearrange("p (t a d) -> p t a d", t=gwc, a=P_OUT),
                    wyx_g[:, :, 0:14][:, :, :, None].broadcast_to(
                        [128, gwc, P_OUT, 2]
                    ),
                )

                def tt(w2_ap, c0, ncc):
                    """w2[p, t, a, b] = wy[p, t, a] * wx[p, t, b] for columns
                    [c0, c0+ncc) of this group's wyx/wyd tiles."""
                    wyd_r = wyd_t[:, c0 * 28 : (c0 + ncc) * 28].rearrange(
                        "p (t a d) -> p t a d", t=ncc, a=P_OUT
                    )
                    wx_pair = wyx_t[:, c0 * 28 : (c0 + ncc) * 28].rearrange(
                        "p (t z) -> p t z", t=ncc
                    )[:, :, 14:28].rearrange("p t (b2 d) -> p t b2 d", d=2)
                    eng = pick(ncc * _TT_V, ncc * _TT_G)
                    (nc.vector if eng == "v" else nc.gpsimd).tensor_mul(
                        w2_ap.rearrange(
                            "p (t a b2 d) -> p t a b2 d", t=ncc, a=P_OUT, b2=7
                        ),
                        wyd_r[:, :, :, None, :].broadcast_to(
                            [128, ncc, P_OUT, 7, 2]
                        ),
                        wx_pair[:, :, None, :, :].broadcast_to(
                            [128, ncc, P_OUT, 7, 2]
                        ),
                    )

                ot = out_pool.tile([128, GROUP * 392], BF16)
                for i, (nf, fstart, tcol, toff, th, wtail) in enumerate(
                    G["slots"]
                ):
                    if nf > 0:
                        w2f = w2_pool.tile([128, nf * 196], BF16, tag="w2")
                        tt(w2f[:], fstart, nf)
                    if th > 0:
                        # tail: dedicated zero-masked wyx column -> full
                        # 128-row contraction against the SHARED regs column
                        w2t = w2_pool.tile([128, 196], BF16, tag="w2")
                        tt(w2t[:], wtail, 1)
                    ps = psum_pool.tile([128, 1024], F32)
                    # channel halves accumulate in DIFFERENT PSUM banks: a
                    # matmul's start=True clears the whole bank, so
                    # interleaved chains sharing a bank corrupt each other
                    for t in range(nf):
                        for h in range(2):
                            nc.tensor.matmul(
                                ps[:, h * 512 : h * 512 + 196],
                                reg_t[
                                    :,
                                    (fstart + t) * 256
                                    + h * 128 : (fstart + t) * 256
                                    + (h + 1) * 128,
                                ],
                                w2f[:, t * 196 : (t + 1) * 196],
                                start=(t == 0),
                                stop=(t == nf - 1 and th == 0),
                            )
                    if th > 0:
                        for h in range(2):
                            nc.tensor.matmul(
                                ps[:, h * 512 : h * 512 + 196],
                                reg_t[
                                    :,
                                    tcol * 256 + h * 128 : tcol * 256
                                    + (h + 1) * 128,
                                ],
                                w2t[:, :],
                                start=(nf == 0),
                                stop=True,
                            )
                    ps_view = ps[:].rearrange("p (h z) -> p h z", h=2)[:, :, 0:196]
                    ot_view = ot[:, i * 392 : (i + 1) * 392].rearrange(
                        "p (h z) -> p h z", h=2
                    )
                    # last group's copies split ACT/DVE so the drain chain
                    # (copies -> final stores) finishes sooner
                    if g == n_groups - 1 and i % 2 == 1:
                        nc.vector.tensor_copy(ot_view, ps_view)
                    else:
                        nc.scalar.copy(ot_view, ps_view)
                # mid stores on the scalar HWDGE ring: they sit right after
                # the copies they depend on in ACT program order, so the
                # sequencer never stalls on them (on the sync ring a store's
                # semaphore wait would block the NEXT group's load
                # descriptor-gen and collapse the pipeline).  Tail stores on
                # sync (idle once loads are done).  2 per group max — each
                # dma_start costs ~0.65us of serial sequencer descriptor-gen.
                if g >= n_groups - 2:
                    half = GROUP // 2 * 392
                    nc.sync.dma_start(out[g][:, :half], ot[:, :half])
                    nc.sync.dma_start(out[g][:, half:], ot[:, half:])
                else:
                    nc.scalar.dma_start(out[g], ot[:])
                start += gcc
    nc.compile()
    _GRAPH_CACHE[layout_key] = nc
    return nc


# ---------------------------------------------------------------- entry point
def _run(feature_f4, feature_f8, boxes, batch_idx, trace=False):
    from concourse.bass_utils import run_bass_kernel_spmd

    feature_f4 = np.ascontiguousarray(np.asarray(feature_f4, dtype=np.float32))
    feature_f8 = np.ascontiguousarray(np.asarray(feature_f8, dtype=np.float32))
    boxes = np.asarray(boxes, dtype=np.float32)
    batch_idx = np.asarray(batch_idx)

    rois = _build_rois(boxes, batch_idx)
    R = len(rois)
    assert R % N_CORES == 0
    n_slots = R // N_CORES
    n_groups = n_slots // GROUP

    # shard: descending size, round-robin deal => slot rank j holds ROIs of
    # rank j*8..j*8+7, so per-slot sizes are near-equal across cores
    order = sorted(range(R), key=lambda r: (-rois[r]["k"], r))
    assign = [order[c::N_CORES] for c in range(N_CORES)]  # [core][rank] -> roi
    # snake-deal slots so every GROUP mixes large and small ROIs: BALANCED
    # group sizes keep the PE continuously fed (skewed groups starve it
    # mid-kernel and trip HAM re-throttling)
    slot_order = [g + n_groups * i for g in range(n_groups) for i in range(GROUP)]
    # order groups: 2nd-smallest first (fast ramp-in), smallest LAST (fast
    # pipeline drain)
    gw = []
    for g in range(n_groups):
        members = slot_order[g * GROUP : (g + 1) * GROUP]
        gw.append((sum(rois[assign[0][s]]["k"] for s in members), g))
    asc = [g for _, g in sorted(gw)]
    g_order = [asc[1]] + asc[2:][::-1] + [asc[0]]
    slot_order = [
        s for g in g_order for s in slot_order[g * GROUP : (g + 1) * GROUP]
    ]
    assign = [[a[s] for s in slot_order] for a in assign]

    slot_k = [
        max(rois[assign[c][j]]["k"] for c in range(N_CORES))
        for j in range(n_slots)
    ]
    layout = _build_layout(slot_k)
    layout_key = tuple(
        (tuple(G["slots"]), G["ncols"], G["wcols"], G["nfull"])
        for G in layout
    )
    tot = sum(G["ncols"] for G in layout)
    totw = sum(G["wcols"] for G in layout)

    # NHWC bf16 feature copies for row gathering
    feats_bf = [
        np.ascontiguousarray(feature_f4.transpose(0, 2, 3, 1)).astype(BF16_NP),
        np.ascontiguousarray(feature_f8.transpose(0, 2, 3, 1)).astype(BF16_NP),
    ]

    in_maps = []
    for c in range(N_CORES):
        regs_c = np.zeros((128, tot, 256), BF16_NP)
        wyxs_c = np.zeros((128, totw, 28), BF16_NP)
        base = 0
        wbase = 0
        for g, G in enumerate(layout):
            for i, (nf, fstart, tcol, toff, th, wtail) in enumerate(
                G["slots"]
            ):
                d = rois[assign[c][g * GROUP + i]]
                k = d["k"]
                vals = np.empty((k, 256), BF16_NP)
                for lvl in (0, 1):
                    sel = d["lvl"] == lvl
                    if sel.any():
                        vals[sel] = feats_bf[lvl][d["b"]][d["y"][sel], d["x"][sel]]
                wy_bf = d["wy"].astype(BF16_NP)
                wx_bf = d["wx"].astype(BF16_NP)

                def put(vcol, wcol, prow, a, b):
                    n = b - a
                    regs_c[prow : prow + n, base + vcol, :] = vals[a:b]
                    wyxs_c[prow : prow + n, wbase + wcol, 0:14] = wy_bf[a:b]
                    wyxs_c[prow : prow + n, wbase + wcol, 14:28] = wx_bf[a:b]

                for t in range(nf):
                    a, b = t * CHUNK_K, min((t + 1) * CHUNK_K, k)
                    if a >= k:
                        break
                    put(fstart + t, fstart + t, 0, a, b)
                if th > 0 and k > nf * CHUNK_K:
                    put(tcol, wtail, toff, nf * CHUNK_K, k)
            base += G["ncols"]
            wbase += G["wcols"]
        in_maps.append({"regs": regs_c, "wyxs": wyxs_c})

    nc = _build_graph(layout_key)
    res = run_bass_kernel_spmd(
        nc, in_maps, core_ids=list(range(N_CORES)), trace=trace
    )

    # unshard + layout fix
    out_full = np.empty((R, 256, P_OUT, P_OUT), np.float32)
    for c in range(N_CORES):
        o = res.results[c]["out"].astype(np.float32)  # [n_groups, 128, G*392]
        o = o.reshape(n_groups, 128, GROUP, 2, 196).transpose(0, 2, 3, 1, 4)
        o = o.reshape(n_slots, 256, 196)
        out_full[assign[c]] = o.reshape(n_slots, 256, P_OUT, P_OUT)
    return out_full, res


def kernel(feature_f4, feature_f8, boxes, batch_idx):
    out, _ = _run(feature_f4, feature_f8, boxes, batch_idx, trace=False)
    return out
